# revision 39
# baseline (speedup 1.0000x reference)
"""Trainium2 Bass kernel for FASTMultiHeadAttention (fastmax, Taylor-2 softmax approx
with relative positional embeddings, optional causal mask).

B=1, H=8, N=2048, D=64. One head per NeuronCore (8 cores).

Math per head (q,k,v: [N,D], rpe: [2N-1, D]):
    s[i,j]  = q_i.k_j + q_i.rpe[i-j+N-1]
    w       = 1 + s + s^2/2      (causal-masked if mask)
    out_i   = sum_j w[i,j] v_j / sum_j w[i,j]

Device algorithm (per head), all-bf16 PE pipeline:
    w = ((s+1)^2 + 1)/2 on valid entries, so with t = (s+1)^2 (t=0 on masked):
      out_i = (sum_j t v + vcum_i) / (sum_j t + (i+1))
    - content scores:  PE matmul (bf16)  S = qT.T @ kT        [i-part, j-free]
    - rpe scores:      PE matmul QR = qT.T @ rpe_rev windows -> evac bf16 ->
      diagonal SBUF->SBUF DMA gather R[a,j] = QR[a, 127-a+j]
    - s1 = (S + 1) + R   scalar_tensor_tensor (DVE/Pool split)
    - causal mask: affine_select fills diag chunk with 0 (Pool)
    - W^T via PE transposes (bf16) -> square on PSUM evac (DVE TT-mult / ACT)
    - O[i,d] = sum_j t v directly in [i,d] layout: W^T chunk stationary, V moving
      (65th V col of ones gives the denominator); batched normalize; contig store
"""

import sys
import os
import numpy as np

for _p in ("/opt/trn_rl_repo", "/root/.axon_site/_ro/trn_rl_repo"):
    if os.path.isdir(_p) and _p not in sys.path:
        sys.path.insert(0, _p)

B, H, N, D = 1, 8, 2048, 64
NT = N // 128            # 16 i-tiles of 128 rows
NJC = N // 128           # 16 j-chunks of 128

_CACHE = {}


def _tile_order(causal):
    """Processing order of i-tiles. "hill": small tiles at both ends so the
    pipeline warms up and drains on cheap tiles."""
    if not causal:
        return list(range(NT))
    mode = TUNE.get("order", "hill")
    if mode == "fwd":
        return list(range(NT))
    if mode == "rev":
        return list(range(NT))[::-1]
    ev = list(range(0, NT, 2))
    od = list(range(1, NT, 2))[::-1]
    return ev + od

# engine-assignment tuning (fractions routed to the listed engine)
TUNE = {
    "qr_act_frac": 0.6,     # QR psum->sbuf copies on ACT (rest DVE)
    "s1_2step_frac": 0.3,   # s1 chunks via ACT evac(+1) then TT-add (else DVE stt)
    "s1_pool_frac": 1.0,    # ... of the 2-step TT-adds, fraction on Pool (rest DVE)
    "sq_act_frac": 1.0,     # square-evac groups on ACT (rest DVE copy + sbuf square)
    "presq_frac": 0.4,      # tiles squared in SBUF pre-transpose (evac = 2x DVE copy)
    "presq_pool_frac": 0.4, # ... of pre-square chunks, fraction on Pool stt (rest DVE TT)
    "tr_group": 8,          # transpose chunks per evac group
    "depth_b": 3,           # pipeline offset of stageB behind stageA
    "order": "hill",        # causal tile processing order
    "load_split": True,     # split first loads to unblock stageA sooner
    "psum_cfg": 1,          # 1: QR2/S1/tr1/ob1  2: QR1/S1/tr2/ob2
}


def _build_program(causal: bool, reps: int = 1):
    import concourse.bass as bass
    from concourse import bacc
    import concourse.mybir as mybir
    from concourse.tile import TileContext
    from concourse.masks import make_identity

    fp32 = mybir.dt.float32
    bf16 = mybir.dt.bfloat16
    AT = mybir.ActivationFunctionType
    OP = mybir.AluOpType

    RPW = 2560 if causal else 4608   # rpe_rev padded width
    QRW = 2176                       # QR buffer width (127 + max j_max)
    # W^T storage: t-major, jc-minor, triangular-packed when causal; the
    # per-(t, C-group) evacuation destination is then a contiguous slice
    _njc = [(t + 1 if causal else NJC) for t in range(NT)]
    WT_OFF = [128 * sum(_njc[:t]) for t in range(NT)]
    WTW = 128 * sum(_njc)

    nc = bacc.Bacc("TRN2", target_bir_lowering=False, debug=False)

    # instruction -> pipeline-stage map for trace analysis
    global STAGE_MAP
    STAGE_MAP = {}
    _stage = {"cur": "init"}
    _orig_next = nc.get_next_instruction_name
    def _wrapped_next():
        nm = _orig_next()
        STAGE_MAP[nm] = _stage["cur"]
        return nm
    nc.get_next_instruction_name = _wrapped_next
    def _mark(lbl):
        _stage["cur"] = lbl

    qT_d = nc.dram_tensor("qT", [64, N], bf16, kind="ExternalInput")
    kT_d = nc.dram_tensor("kT", [64, N], bf16, kind="ExternalInput")
    v_d = nc.dram_tensor("vr", [128, NJC * 65], bf16, kind="ExternalInput")
    vcum_d = nc.dram_tensor("vcum", [128, NT * 64], fp32, kind="ExternalInput")
    rpe_d = nc.dram_tensor("rpeT", [64, RPW], bf16, kind="ExternalInput")
    iota_d = nc.dram_tensor("iota", [128, NT], fp32, kind="ExternalInput")
    o_d = nc.dram_tensor("o", [128, NT * 64], fp32, kind="ExternalOutput")

    with TileContext(nc) as tc:
        with (
            tc.tile_pool(name="persist", bufs=1) as pp,
            tc.tile_pool(name="qr", bufs=3) as qrp,
            tc.tile_pool(name="rows", bufs=6) as rp,
            tc.tile_pool(name="small", bufs=2) as sp,
        ):
            class Frac:
                # weighted deterministic router: pick() True with rate `frac`
                def __init__(self, frac):
                    self.f = frac
                    self.acc = 0.0
                def pick(self):
                    self.acc += self.f
                    if self.acc >= 0.999:
                        self.acc -= 1.0
                        return True
                    return False

            qr_act = Frac(TUNE["qr_act_frac"])
            s1_2step = Frac(TUNE["s1_2step_frac"])
            s1_pool = Frac(TUNE["s1_pool_frac"])
            sq_act = Frac(TUNE["sq_act_frac"])
            presq = Frac(TUNE["presq_frac"])
            presq_pool = Frac(TUNE["presq_pool_frac"])
            TRG = TUNE["tr_group"]

            # ---- persistent loads (bf16 inputs, single copy: Ldweights is
            # free so no row-group packing / partition duplication) ----
            qT_s = pp.tile([64, N], bf16, name="qT_s")
            kT_s = pp.tile([64, N], bf16, name="kT_s")
            rpe_s = pp.tile([64, RPW], bf16, name="rpe_s")
            v_s = pp.tile([128, NJC * 65], bf16, name="v_s")
            vcum_s = pp.tile([128, NT * 64], fp32, name="vcum_s")
            iota_s = pp.tile([128, NT], fp32, name="iota_s")
            # load order matters: the first processed tile's qT slice + rpe
            # gate stageA, kT gates stageB; v/vcum/iota are needed much later
            # (scalar ring keeps SP free). Causal processes tiles biggest-
            # first (t=NT-1 down) so the pipeline drain tile is tiny.
            if causal and TUNE.get("load_split", True):
                # first processed tiles (0,2,4) gate on qT[:,0:640] and
                # rpe[:,1408:2560]; the rest streams behind
                nc.sync.dma_start(out=qT_s[:, 0:640],
                                  in_=bass.AP(qT_d.ap().tensor, 0, [[N, 64], [1, 640]]))
                nc.scalar.dma_start(out=rpe_s[:, 1408:RPW],
                                    in_=bass.AP(rpe_d.ap().tensor, 1408, [[RPW, 64], [1, RPW - 1408]]))
                nc.sync.dma_start(out=rpe_s[:, 0:1408],
                                  in_=bass.AP(rpe_d.ap().tensor, 0, [[RPW, 64], [1, 1408]]))
                nc.sync.dma_start(out=qT_s[:, 640:N],
                                  in_=bass.AP(qT_d.ap().tensor, 640, [[N, 64], [1, N - 640]]))
            else:
                nc.sync.dma_start(out=qT_s[:], in_=qT_d.ap())
                nc.sync.dma_start(out=rpe_s[:], in_=rpe_d.ap())
            nc.sync.dma_start(out=kT_s[:], in_=kT_d.ap())
            nc.sync.dma_start(out=v_s[:], in_=v_d.ap())

            ident = pp.tile([128, 128], bf16, name="ident")
            make_identity(nc, ident[:])

            wt_all = pp.tile([128, WTW], bf16, name="wt_all")
            out_s = pp.tile([128, NT * 64], fp32, name="out_s")

            for _rep in range(reps):
              cfg = TUNE.get("psum_cfg", 1)
              qr_bufs, s_bufs, tr_bufs, ob_bufs = (
                  (2, 1, 1, 1) if cfg == 1 else (1, 1, 2, 2))
              with (
                  tc.tile_pool(name="mm_ps", bufs=s_bufs, space="PSUM") as mmp,
                  tc.tile_pool(name="qr_ps", bufs=qr_bufs, space="PSUM") as qmp,
                  tc.tile_pool(name="tr_ps", bufs=tr_bufs, space="PSUM") as trp,
                  tc.tile_pool(name="o_ps", bufs=ob_bufs, space="PSUM") as obp,
              ):
                live = {}

                def stageA(t):
                    # rpe projection QR + diagonal gather of R
                    _mark(f"A{t}")
                    i0 = 128 * t
                    j_max = i0 + 128 if causal else N
                    u_min = (N - 1) - i0 - 127
                    qr_w = 127 + j_max
                    qrbuf = qrp.tile([128, QRW], bf16, name="qrbuf")
                    for b0 in range(0, qr_w, 1024):
                        bw = min(1024, qr_w - b0)
                        qr_ps = qmp.tile([128, 1024], fp32, name="qr_ps")
                        for h0 in range(0, bw, 512):
                            hw = min(512, bw - h0)
                            nc.tensor.matmul(qr_ps[:, h0:h0 + hw],
                                             qT_s[:, i0:i0 + 128],
                                             rpe_s[:, u_min + b0 + h0:u_min + b0 + h0 + hw],
                                             start=True, stop=True)
                        if qr_act.pick():
                            nc.scalar.activation(qrbuf[:, b0:b0 + bw], qr_ps[:, 0:bw],
                                                 AT.Copy, bias=0.0, scale=1.0)
                        else:
                            nc.vector.tensor_copy(qrbuf[:, b0:b0 + bw], qr_ps[:, 0:bw])
                    # diagonal gather R[a, j] = qrbuf[a, 127 - a + j]
                    R_row = rp.tile([128, N], bf16, name="R_row", tag="R_row")
                    diag = bass.AP(qrbuf[:].tensor, qrbuf[:].offset + 127,
                                   [[QRW - 1, 128], [1, j_max]])
                    nc.sync.dma_start(out=R_row[:, 0:j_max], in_=diag)
                    live[t] = R_row

                def stageB(t):
                    # content scores + s1 = (S+1) + R + causal mask
                    _mark(f"B{t}")
                    i0 = 128 * t
                    j_max = i0 + 128 if causal else N
                    R_row = live[t]
                    s1_row = rp.tile([128, N], bf16, name="s1_row", tag="s1_row")
                    for jb in range(0, j_max, 1024):
                        cw = min(1024, j_max - jb)
                        mm_ps = mmp.tile([128, 1024], fp32, name="mm_ps")
                        for h0 in range(0, cw, 512):
                            hw = min(512, cw - h0)
                            nc.tensor.matmul(mm_ps[:, h0:h0 + hw],
                                             qT_s[:, i0:i0 + 128],
                                             kT_s[:, jb + h0:jb + h0 + hw],
                                             start=True, stop=True)
                        if s1_2step.pick():
                            # ACT evacuates S+1 to SBUF; the +R add runs on
                            # Pool/DVE from SBUF (Pool cannot read PSUM)
                            cs_bf = rp.tile([128, 1024], bf16, name="cs_bf", tag="cs_bf")
                            nc.scalar.activation(cs_bf[:, 0:cw], mm_ps[:, 0:cw],
                                                 AT.Copy, bias=1.0, scale=1.0)
                            eng = nc.gpsimd if s1_pool.pick() else nc.vector
                            eng.tensor_tensor(
                                out=s1_row[:, jb:jb + cw], in0=cs_bf[:, 0:cw],
                                in1=R_row[:, jb:jb + cw], op=OP.add)
                        else:
                            nc.vector.scalar_tensor_tensor(
                                out=s1_row[:, jb:jb + cw], in0=mm_ps[:, 0:cw], scalar=1.0,
                                in1=R_row[:, jb:jb + cw], op0=OP.add, op1=OP.add)
                    pre = presq.pick()
                    if pre:
                        # square in SBUF now; transposed PSUM evac becomes a
                        # 2x DVE copy and ACT is relieved
                        t_row = rp.tile([128, N], bf16, name="t_row", tag="t_row")
                        for jb in range(0, j_max, 1024):
                            cw = min(1024, j_max - jb)
                            eng = nc.gpsimd if presq_pool.pick() else nc.vector
                            eng.tensor_tensor(
                                out=t_row[:, jb:jb + cw], in0=s1_row[:, jb:jb + cw],
                                in1=s1_row[:, jb:jb + cw], op=OP.mult)
                        s1_row = t_row
                    s1_diag = None
                    if causal:
                        # masked diagonal chunk goes to its own tile so the mask
                        # doesn't gate the other chunks' transposes
                        s1_diag = rp.tile([128, 128], bf16, name="s1_diag", tag="s1_diag")
                        nc.gpsimd.affine_select(
                            out=s1_diag[:], in_=s1_row[:, i0:i0 + 128],
                            compare_op=OP.is_ge, fill=0.0,
                            base=0, channel_multiplier=1, pattern=[[-1, 128]])
                    live[t] = (s1_diag, s1_row, pre)

                def stageC_group(t, g0):
                    # transpose s1 chunks, square during PSUM evacuation:
                    # wt_all[:, N*jc + i0 : +128] = (s1^T)^2
                    _mark(f"C{t}.{g0 // max(1, TUNE['tr_group'])}")
                    i0 = 128 * t
                    j_max = i0 + 128 if causal else N
                    s1_diag, s1_row, pre = live[t]
                    njc = (j_max + 127) // 128
                    if g0 >= njc:
                        return
                    gn = min(TRG, njc - g0)
                    tr_ps = trp.tile([128, 128 * TRG], bf16, name="tr_ps")
                    for g in range(gn):
                        jc = g0 + g
                        src_chunk = (s1_diag[:] if (causal and jc == t)
                                     else s1_row[:, 128 * jc:128 * (jc + 1)])
                        nc.tensor.transpose(tr_ps[:, 128 * g:128 * (g + 1)],
                                            src_chunk, ident[:])
                    dst = wt_all[:, WT_OFF[t] + 128 * g0:WT_OFF[t] + 128 * (g0 + gn)]
                    srcap = tr_ps[:, 0:128 * gn]
                    if pre:
                        nc.vector.tensor_copy(dst, srcap)
                    elif sq_act.pick():
                        nc.scalar.activation(dst, srcap, AT.Square, bias=0.0, scale=1.0)
                    else:
                        # DVE cannot square from PSUM (one-PSUM-input rule):
                        # 2x copy to SBUF, then all-SBUF square
                        sq_bf = rp.tile([128, 128 * TRG], bf16, name="sq_bf", tag="sq_bf")
                        nc.vector.tensor_copy(sq_bf[:, 0:128 * gn], tr_ps[:, 0:128 * gn])
                        s2 = sq_bf[:, 0:128 * gn]
                        nc.vector.tensor_tensor(out=dst, in0=s2, in1=s2, op=OP.mult)

                def stageO(t, p):
                    # output matmuls: W^T chunk stationary, V moving ->
                    # O accumulates in [i-part, 65] PSUM (col 64 = denominator).
                    # iota/vcum/out are laid out by processing position p.
                    _mark(f"O{t}")
                    i0 = 128 * t
                    u = p % 4
                    if u == 0:
                        live["ob"] = obp.tile([128, 260], fp32, name="ob")
                    ob = live["ob"]
                    jc_hi = t + 1 if causal else NJC
                    for jc in range(jc_hi):
                        nc.tensor.matmul(ob[:, 65 * u:65 * u + 65],
                                         wt_all[:, WT_OFF[t] + 128 * jc:WT_OFF[t] + 128 * jc + 128],
                                         v_s[:, 65 * jc:65 * jc + 65],
                                         start=(jc == 0), stop=(jc == jc_hi - 1))
                    if u == 3:
                        p0 = 4 * (p // 4)
                        # denom = psum col 64 (+65 stride) + iota; recip; then
                        # out = (num + vcum) * recip
                        dcol = bass.AP(ob[:].tensor, ob[:].offset + 64,
                                       [[260, 128], [65, 4]])
                        dtot = sp.tile([128, 4], fp32, name="dtot", tag="dtot")
                        nc.vector.tensor_tensor(out=dtot[:], in0=dcol,
                                                in1=iota_s[:, p0:p0 + 4], op=OP.add)
                        recip = sp.tile([128, 4], fp32, name="recip", tag="recip")
                        nc.vector.reciprocal(recip[:], dtot[:])
                        onum = bass.AP(ob[:].tensor, ob[:].offset,
                                       [[260, 128], [65, 4], [1, 64]])
                        osl = out_s[:, 64 * p0:64 * p0 + 256].rearrange(
                            "p (t d) -> p t d", d=64)
                        nc.vector.tensor_tensor(
                            out=osl, in0=onum,
                            in1=vcum_s[:, 64 * p0:64 * p0 + 256].rearrange(
                                "p (t d) -> p t d", d=64),
                            op=OP.add)
                        rb = bass.AP(recip[:].tensor, recip[:].offset,
                                     [[4, 128], [1, 4], [0, 64]])
                        nc.vector.tensor_tensor(out=osl, in0=osl, in1=rb, op=OP.mult)
                        dstap = bass.AP(o_d.ap().tensor, 64 * p0,
                                        [[NT * 64, 128], [1, 256]])
                        nc.sync.dma_start(out=dstap, in_=out_s[:, 64 * p0:64 * p0 + 256])

                # per-u issue order: oldest dependencies first so no engine
                # queue head-of-line blocks on freshly-issued producers; the
                # two transpose groups of C(t) are separated by B's matmuls so
                # the single trp buffer recycles without stalling PE.
                # Causal runs tiles biggest-first: the drain tile is tiny.
                NG = (NJC + TRG - 1) // TRG
                order = _tile_order(causal)
                DB = TUNE["depth_b"]
                DC, DO = DB + 1, DB + 2
                body = TUNE.get("body_order", 0)
                for u in range(NT + DO):
                    if u == 6:
                        nc.scalar.dma_start(out=vcum_s[:], in_=vcum_d.ap())
                        nc.scalar.dma_start(out=iota_s[:], in_=iota_d.ap())
                    def doA():
                        if u < NT:
                            stageA(order[u])
                    def doB():
                        if DB <= u < NT + DB:
                            stageB(order[u - DB])
                    def doC0():
                        if DC <= u < NT + DC:
                            stageC_group(order[u - DC], 0)
                    def doC1():
                        if DC <= u < NT + DC:
                            for g in range(1, NG):
                                stageC_group(order[u - DC], TRG * g)
                            live.pop(order[u - DC])
                    def doO():
                        if DO <= u < NT + DO:
                            stageO(order[u - DO], u - DO)
                    seqs = {
                        0: (doO, doC0, doB, doC1, doA),
                        1: (doA, doO, doC0, doB, doC1),
                        2: (doA, doB, doC0, doC1, doO),
                        3: (doA, doO, doC0, doC1, doB),
                        4: (doO, doA, doC0, doB, doC1),
                    }
                    for fn in seqs[body]:
                        fn()

    nc.compile()
    return nc


def _make_runner(nc, n_cores):
    import concourse.mybir as mybir
    import jax
    from jax.sharding import Mesh, PartitionSpec
    from jax.experimental.shard_map import shard_map
    from concourse.bass2jax import install_neuronx_cc_hook, _bass_exec_p, partition_id_tensor

    install_neuronx_cc_hook()
    partition_name = nc.partition_id_tensor.name if nc.partition_id_tensor else None
    in_names, out_names, out_avals, zero_outs = [], [], [], []
    for alloc in nc.m.functions[0].allocations:
        if not isinstance(alloc, mybir.MemoryLocationSet):
            continue
        name = alloc.memorylocations[0].name
        if alloc.kind == "ExternalInput":
            if name != partition_name:
                in_names.append(name)
        elif alloc.kind == "ExternalOutput":
            shape = tuple(alloc.tensor_shape)
            dtype = mybir.dt.np(alloc.dtype)
            out_names.append(name)
            out_avals.append(jax.core.ShapedArray(shape, dtype))
            zero_outs.append(np.zeros(shape, dtype))
    n_params = len(in_names)
    n_outs = len(out_avals)
    all_in_names = list(in_names) + list(out_names)
    if partition_name is not None:
        all_in_names.append(partition_name)

    def _body(*args):
        operands = list(args)
        if partition_name is not None:
            operands.append(partition_id_tensor())
        outs = _bass_exec_p.bind(
            *operands, out_avals=tuple(out_avals), in_names=tuple(all_in_names),
            out_names=tuple(out_names), lowering_input_output_aliases=(),
            sim_require_finite=True, sim_require_nnan=True, nc=nc)
        return tuple(outs)

    devices = jax.devices()[:n_cores]
    mesh = Mesh(np.asarray(devices), ("core",))
    in_specs = (PartitionSpec("core"),) * (n_params + n_outs)
    out_specs = (PartitionSpec("core"),) * n_outs
    jitted = jax.jit(shard_map(_body, mesh=mesh, in_specs=in_specs,
                               out_specs=out_specs, check_rep=False), keep_unused=True)

    def run(in_maps):
        concat_in = [np.concatenate([np.asarray(in_maps[c][n]) for c in range(n_cores)], axis=0)
                     for n in in_names]
        concat_zeros = [np.zeros((n_cores * z.shape[0], *z.shape[1:]), z.dtype) for z in zero_outs]
        outs = jitted(*concat_in, *concat_zeros)
        import jax as _jax
        _jax.block_until_ready(outs)
        return [{name: np.asarray(outs[i]).reshape(n_cores, *out_avals[i].shape)[c]
                 for i, name in enumerate(out_names)} for c in range(n_cores)]
    return run


def _get_runner(causal: bool):
    key = bool(causal)
    if key not in _CACHE:
        nc = _build_program(key)
        _CACHE[key] = _make_runner(nc, H)
    return _CACHE[key]


def _bf16(x):
    import ml_dtypes
    return np.ascontiguousarray(x).astype(ml_dtypes.bfloat16)


def _prep_head(q2, k2, v2, causal):
    """q2,k2,v2: [N, D] fp32 for one head. Returns per-core input dict."""
    qT = _bf16(q2.T)                                     # [64, N]
    kT = _bf16(k2.T)
    # v with ones col 64 (denominator)
    v3 = np.concatenate([v2, np.ones((N, 1), np.float32)], axis=1)  # [N, 65]
    v_r = _bf16(v3.reshape(NJC, 128, 65).transpose(1, 0, 2).reshape(128, NJC * 65))
    if causal:
        vc = np.cumsum(v2, axis=0, dtype=np.float64).astype(np.float32)
    else:
        vc = np.broadcast_to(v2.sum(axis=0, dtype=np.float64).astype(np.float32), (N, 64))
    vcum = vc.reshape(NT, 128, 64)[_tile_order(causal)]
    vcum = np.ascontiguousarray(vcum.transpose(1, 0, 2).reshape(128, NT * 64))
    return {"qT": qT, "kT": kT, "vr": v_r, "vcum": vcum}


def kernel(q, k, v, rpe_matrix, mask):
    causal = bool(np.asarray(mask).item()) if not isinstance(mask, (int, bool)) else bool(mask)
    q = np.asarray(q, dtype=np.float32)
    k = np.asarray(k, dtype=np.float32)
    v = np.asarray(v, dtype=np.float32)
    rpe = np.asarray(rpe_matrix, dtype=np.float32)

    RPW = 2560 if causal else 4608
    if causal:
        # u in [0, N-1]: rpe_rev[u] = rpe[2N-2-u] -> rows 2N-2 .. N-1
        rpe_rev = rpe[N - 1:2 * N - 1][::-1]             # [N, 64]
    else:
        rpe_rev = rpe[::-1]                              # [2N-1, 64]
    rpeT = np.zeros((64, RPW), dtype=np.float32)
    rpeT[:, :rpe_rev.shape[0]] = rpe_rev.T
    rpeT = _bf16(rpeT)

    a = np.arange(128, dtype=np.float32)[:, None]
    tt = np.asarray(_tile_order(causal), dtype=np.float32)[None, :]
    iota = (128 * tt + a + 1.0) if causal else np.full((128, NT), float(N), np.float32)
    iota = np.ascontiguousarray(iota.astype(np.float32))

    run = _get_runner(causal)
    in_maps = []
    for h in range(H):
        m = _prep_head(q[0, h], k[0, h], v[0, h], causal)
        m["rpeT"] = rpeT
        m["iota"] = iota
        in_maps.append(m)
    results = run(in_maps)
    # o_d layout: o[a, 64*p + d] = out[128*order[p] + a, d]
    inv = np.argsort(np.asarray(_tile_order(causal)))
    out = np.stack([
        results[h]["o"].reshape(128, NT, 64)[:, inv].transpose(1, 0, 2).reshape(N, 64)
        for h in range(H)])[None]
    return out.astype(np.float32)


if __name__ == "__main__":
    rng = np.random.default_rng(0)
    q = rng.standard_normal((B, H, N, D), dtype=np.float32)
    k = rng.standard_normal((B, H, N, D), dtype=np.float32)
    v = rng.standard_normal((B, H, N, D), dtype=np.float32)
    rpe = rng.standard_normal((2 * N - 1, D), dtype=np.float32)
    o = kernel(q, k, v, rpe, 1)
    print("out", o.shape, o.dtype, np.abs(o).mean())


# revision 45
# speedup vs baseline: 1.0965x; 1.0965x over previous
"""Trainium2 Bass kernel for FASTMultiHeadAttention (fastmax, Taylor-2 softmax approx
with relative positional embeddings, optional causal mask).

B=1, H=8, N=2048, D=64. One head per NeuronCore (8 cores).

Math per head (q,k,v: [N,D], rpe: [2N-1, D]):
    s[i,j]  = q_i.k_j + q_i.rpe[i-j+N-1]
    w       = 1 + s + s^2/2      (causal-masked if mask)
    out_i   = sum_j w[i,j] v_j / sum_j w[i,j]

Device algorithm (per head), all-bf16 PE pipeline:
    w = ((s+1)^2 + 1)/2 on valid entries, so with t = (s+1)^2 (t=0 on masked):
      out_i = (sum_j t v + vcum_i) / (sum_j t + (i+1))
    - content scores:  PE matmul (bf16)  S = qT.T @ kT        [i-part, j-free]
    - rpe scores:      PE matmul QR = qT.T @ rpe_rev windows -> evac bf16 ->
      diagonal SBUF->SBUF DMA gather R[a,j] = QR[a, 127-a+j]
    - s1 = (S + 1) + R   scalar_tensor_tensor (DVE/Pool split)
    - causal mask: affine_select fills diag chunk with 0 (Pool)
    - W^T via PE transposes (bf16) -> square on PSUM evac (DVE TT-mult / ACT)
    - O[i,d] = sum_j t v directly in [i,d] layout: W^T chunk stationary, V moving
      (65th V col of ones gives the denominator); batched normalize; contig store
"""

import sys
import os
import numpy as np

for _p in ("/opt/trn_rl_repo", "/root/.axon_site/_ro/trn_rl_repo"):
    if os.path.isdir(_p) and _p not in sys.path:
        sys.path.insert(0, _p)

B, H, N, D = 1, 8, 2048, 64
NT = N // 128            # 16 i-tiles of 128 rows
NJC = N // 128           # 16 j-chunks of 128

_CACHE = {}


def _tile_order(causal):
    """Processing order of i-tiles. "hill": small tiles at both ends so the
    pipeline warms up and drains on cheap tiles."""
    if not causal:
        return list(range(NT))
    mode = TUNE.get("order", "hill")
    if mode == "fwd":
        return list(range(NT))
    if mode == "rev":
        return list(range(NT))[::-1]
    ev = list(range(0, NT, 2))
    od = list(range(1, NT, 2))[::-1]
    return ev + od

# engine-assignment tuning (fractions routed to the listed engine)
TUNE = {
    "qr_act_frac": 0.65,    # QR psum->sbuf copies on ACT (rest DVE)
    "s1_2step_frac": 0.0,   # s1 chunks via ACT evac(+1) then TT-add (else DVE stt)
    "s1_pool_frac": 1.0,    # ... of the 2-step TT-adds, fraction on Pool (rest DVE)
    "sq_act_frac": 1.0,     # square-evac groups on ACT (rest DVE copy + sbuf square)
    "presq_frac": 0.0,      # tiles squared in SBUF pre-transpose (evac = 2x DVE copy)
    "presq_pool_frac": 0.5, # ... of pre-square chunks, fraction on Pool (rest DVE TT)
    "tr_group": 8,          # transpose chunks per evac group
    "depth_b": 2,           # pipeline offset of stageB behind stageA
    "body_order": 2,        # per-u issue order variant
    "qr_bufs": 4,           # qrbuf SBUF ring depth
    "rows_bufs": 8,         # R_row/s1_row/... SBUF ring depth
    "order": "hill",        # causal tile processing order
    "load_split": True,     # split first loads to unblock stageA sooner
    "psum_cfg": 1,          # 1: QR2/S1/tr1/ob1  2: QR1/S1/tr2/ob2
}


def _build_program(causal: bool, reps: int = 1):
    import concourse.bass as bass
    from concourse import bacc
    import concourse.mybir as mybir
    from concourse.tile import TileContext
    from concourse.masks import make_identity

    fp32 = mybir.dt.float32
    bf16 = mybir.dt.bfloat16
    AT = mybir.ActivationFunctionType
    OP = mybir.AluOpType

    RPW = 2560 if causal else 4608   # rpe_rev padded width
    QRW = 2176                       # QR buffer width (127 + max j_max)
    # W^T storage: t-major, jc-minor, triangular-packed when causal; the
    # per-(t, C-group) evacuation destination is then a contiguous slice
    _njc = [(t + 1 if causal else NJC) for t in range(NT)]
    WT_OFF = [128 * sum(_njc[:t]) for t in range(NT)]
    WTW = 128 * sum(_njc)

    nc = bacc.Bacc("TRN2", target_bir_lowering=False, debug=False)

    # instruction -> pipeline-stage map for trace analysis
    global STAGE_MAP
    STAGE_MAP = {}
    _stage = {"cur": "init"}
    _orig_next = nc.get_next_instruction_name
    def _wrapped_next():
        nm = _orig_next()
        STAGE_MAP[nm] = _stage["cur"]
        return nm
    nc.get_next_instruction_name = _wrapped_next
    def _mark(lbl):
        _stage["cur"] = lbl

    qT_d = nc.dram_tensor("qT", [64, N], bf16, kind="ExternalInput")
    kT_d = nc.dram_tensor("kT", [64, N], bf16, kind="ExternalInput")
    v_d = nc.dram_tensor("vr", [128, NJC * 65], bf16, kind="ExternalInput")
    vcum_d = nc.dram_tensor("vcum", [128, NT * 64], fp32, kind="ExternalInput")
    rpe_d = nc.dram_tensor("rpeT", [64, RPW], bf16, kind="ExternalInput")
    iota_d = nc.dram_tensor("iota", [128, NT], fp32, kind="ExternalInput")
    o_d = nc.dram_tensor("o", [128, NT * 64], fp32, kind="ExternalOutput")

    with TileContext(nc) as tc:
        with (
            tc.tile_pool(name="persist", bufs=1) as pp,
            tc.tile_pool(name="qr", bufs=TUNE.get("qr_bufs", 3)) as qrp,
            tc.tile_pool(name="rows", bufs=TUNE.get("rows_bufs", 6)) as rp,
            tc.tile_pool(name="small", bufs=2) as sp,
        ):
            class Frac:
                # weighted deterministic router: pick() True with rate `frac`
                def __init__(self, frac):
                    self.f = frac
                    self.acc = 0.0
                def pick(self):
                    self.acc += self.f
                    if self.acc >= 0.999:
                        self.acc -= 1.0
                        return True
                    return False

            qr_act = Frac(TUNE["qr_act_frac"])
            s1_2step = Frac(TUNE["s1_2step_frac"])
            s1_pool = Frac(TUNE["s1_pool_frac"])
            sq_act = Frac(TUNE["sq_act_frac"])
            presq = Frac(TUNE["presq_frac"])
            presq_pool = Frac(TUNE["presq_pool_frac"])
            TRG = TUNE["tr_group"]

            # ---- persistent loads (bf16 inputs, single copy: Ldweights is
            # free so no row-group packing / partition duplication) ----
            qT_s = pp.tile([64, N], bf16, name="qT_s")
            kT_s = pp.tile([64, N], bf16, name="kT_s")
            rpe_s = pp.tile([64, RPW], bf16, name="rpe_s")
            v_s = pp.tile([128, NJC * 65], bf16, name="v_s")
            vcum_s = pp.tile([128, NT * 64], fp32, name="vcum_s")
            iota_s = pp.tile([128, NT], fp32, name="iota_s")
            # load order matters: the first processed tile's qT slice + rpe
            # gate stageA, kT gates stageB; v/vcum/iota are needed much later
            # (scalar ring keeps SP free). Causal processes tiles biggest-
            # first (t=NT-1 down) so the pipeline drain tile is tiny.
            if causal and TUNE.get("load_split", True):
                # first processed tiles (0,2,4) gate on qT[:,0:640] and
                # rpe[:,1408:2560]; the rest streams behind
                nc.sync.dma_start(out=qT_s[:, 0:640],
                                  in_=bass.AP(qT_d.ap().tensor, 0, [[N, 64], [1, 640]]))
                nc.scalar.dma_start(out=rpe_s[:, 1408:RPW],
                                    in_=bass.AP(rpe_d.ap().tensor, 1408, [[RPW, 64], [1, RPW - 1408]]))
                nc.sync.dma_start(out=rpe_s[:, 0:1408],
                                  in_=bass.AP(rpe_d.ap().tensor, 0, [[RPW, 64], [1, 1408]]))
                nc.sync.dma_start(out=qT_s[:, 640:N],
                                  in_=bass.AP(qT_d.ap().tensor, 640, [[N, 64], [1, N - 640]]))
            else:
                nc.sync.dma_start(out=qT_s[:], in_=qT_d.ap())
                nc.sync.dma_start(out=rpe_s[:], in_=rpe_d.ap())
            nc.sync.dma_start(out=kT_s[:], in_=kT_d.ap())
            nc.sync.dma_start(out=v_s[:], in_=v_d.ap())

            ident = pp.tile([128, 128], bf16, name="ident")
            make_identity(nc, ident[:])

            wt_all = pp.tile([128, WTW], bf16, name="wt_all")
            out_s = pp.tile([128, NT * 64], fp32, name="out_s")

            for _rep in range(reps):
              cfg = TUNE.get("psum_cfg", 1)
              qr_psb, s_psb, tr_bufs, ob_bufs = (
                  (2, 1, 1, 1) if cfg == 1 else (1, 1, 2, 2))
              with (
                  tc.tile_pool(name="mm_ps", bufs=s_psb, space="PSUM") as mmp,
                  tc.tile_pool(name="qr_ps", bufs=qr_psb, space="PSUM") as qmp,
                  tc.tile_pool(name="tr_ps", bufs=tr_bufs, space="PSUM") as trp,
                  tc.tile_pool(name="o_ps", bufs=ob_bufs, space="PSUM") as obp,
              ):
                live = {}

                def stageA(t):
                    # rpe projection QR + diagonal gather of R
                    _mark(f"A{t}")
                    i0 = 128 * t
                    j_max = i0 + 128 if causal else N
                    u_min = (N - 1) - i0 - 127
                    qr_w = 127 + j_max
                    qrbuf = qrp.tile([128, QRW], bf16, name="qrbuf")
                    for b0 in range(0, qr_w, 1024):
                        bw = min(1024, qr_w - b0)
                        qr_ps = qmp.tile([128, 1024], fp32, name="qr_ps")
                        for h0 in range(0, bw, 512):
                            hw = min(512, bw - h0)
                            nc.tensor.matmul(qr_ps[:, h0:h0 + hw],
                                             qT_s[:, i0:i0 + 128],
                                             rpe_s[:, u_min + b0 + h0:u_min + b0 + h0 + hw],
                                             start=True, stop=True)
                        if qr_act.pick():
                            nc.scalar.activation(qrbuf[:, b0:b0 + bw], qr_ps[:, 0:bw],
                                                 AT.Copy, bias=0.0, scale=1.0)
                        else:
                            nc.vector.tensor_copy(qrbuf[:, b0:b0 + bw], qr_ps[:, 0:bw])
                    # diagonal gather R[a, j] = qrbuf[a, 127 - a + j]
                    R_row = rp.tile([128, N], bf16, name="R_row", tag="R_row")
                    diag = bass.AP(qrbuf[:].tensor, qrbuf[:].offset + 127,
                                   [[QRW - 1, 128], [1, j_max]])
                    nc.sync.dma_start(out=R_row[:, 0:j_max], in_=diag)
                    live[t] = R_row

                def stageB(t):
                    # content scores + s1 = (S+1) + R + causal mask
                    _mark(f"B{t}")
                    i0 = 128 * t
                    j_max = i0 + 128 if causal else N
                    R_row = live[t]
                    s1_row = rp.tile([128, N], bf16, name="s1_row", tag="s1_row")
                    for jb in range(0, j_max, 1024):
                        cw = min(1024, j_max - jb)
                        mm_ps = mmp.tile([128, 1024], fp32, name="mm_ps")
                        for h0 in range(0, cw, 512):
                            hw = min(512, cw - h0)
                            nc.tensor.matmul(mm_ps[:, h0:h0 + hw],
                                             qT_s[:, i0:i0 + 128],
                                             kT_s[:, jb + h0:jb + h0 + hw],
                                             start=True, stop=True)
                        if s1_2step.pick():
                            # ACT evacuates S+1 to SBUF; the +R add runs on
                            # Pool/DVE from SBUF (Pool cannot read PSUM)
                            cs_bf = rp.tile([128, 1024], bf16, name="cs_bf", tag="cs_bf")
                            nc.scalar.activation(cs_bf[:, 0:cw], mm_ps[:, 0:cw],
                                                 AT.Copy, bias=1.0, scale=1.0)
                            eng = nc.gpsimd if s1_pool.pick() else nc.vector
                            eng.tensor_tensor(
                                out=s1_row[:, jb:jb + cw], in0=cs_bf[:, 0:cw],
                                in1=R_row[:, jb:jb + cw], op=OP.add)
                        else:
                            nc.vector.scalar_tensor_tensor(
                                out=s1_row[:, jb:jb + cw], in0=mm_ps[:, 0:cw], scalar=1.0,
                                in1=R_row[:, jb:jb + cw], op0=OP.add, op1=OP.add)
                    pre = presq.pick()
                    if pre:
                        # square in SBUF now; transposed PSUM evac becomes a
                        # 2x DVE copy and ACT is relieved
                        t_row = rp.tile([128, N], bf16, name="t_row", tag="t_row")
                        for jb in range(0, j_max, 1024):
                            cw = min(1024, j_max - jb)
                            eng = nc.gpsimd if presq_pool.pick() else nc.vector
                            eng.tensor_tensor(
                                out=t_row[:, jb:jb + cw], in0=s1_row[:, jb:jb + cw],
                                in1=s1_row[:, jb:jb + cw], op=OP.mult)
                        s1_row = t_row
                    s1_diag = None
                    if causal:
                        # masked diagonal chunk goes to its own tile so the mask
                        # doesn't gate the other chunks' transposes
                        s1_diag = rp.tile([128, 128], bf16, name="s1_diag", tag="s1_diag")
                        nc.gpsimd.affine_select(
                            out=s1_diag[:], in_=s1_row[:, i0:i0 + 128],
                            compare_op=OP.is_ge, fill=0.0,
                            base=0, channel_multiplier=1, pattern=[[-1, 128]])
                    live[t] = (s1_diag, s1_row, pre)

                def stageC_group(t, g0):
                    # transpose s1 chunks, square during PSUM evacuation:
                    # wt_all[:, N*jc + i0 : +128] = (s1^T)^2
                    _mark(f"C{t}.{g0 // max(1, TUNE['tr_group'])}")
                    i0 = 128 * t
                    j_max = i0 + 128 if causal else N
                    s1_diag, s1_row, pre = live[t]
                    njc = (j_max + 127) // 128
                    if g0 >= njc:
                        return
                    gn = min(TRG, njc - g0)
                    tr_ps = trp.tile([128, 128 * TRG], bf16, name="tr_ps")
                    for g in range(gn):
                        jc = g0 + g
                        src_chunk = (s1_diag[:] if (causal and jc == t)
                                     else s1_row[:, 128 * jc:128 * (jc + 1)])
                        nc.tensor.transpose(tr_ps[:, 128 * g:128 * (g + 1)],
                                            src_chunk, ident[:])
                    dst = wt_all[:, WT_OFF[t] + 128 * g0:WT_OFF[t] + 128 * (g0 + gn)]
                    srcap = tr_ps[:, 0:128 * gn]
                    if pre:
                        nc.vector.tensor_copy(dst, srcap)
                    elif sq_act.pick():
                        nc.scalar.activation(dst, srcap, AT.Square, bias=0.0, scale=1.0)
                    else:
                        # DVE cannot square from PSUM (one-PSUM-input rule):
                        # 2x copy to SBUF, then all-SBUF square
                        sq_bf = rp.tile([128, 128 * TRG], bf16, name="sq_bf", tag="sq_bf")
                        nc.vector.tensor_copy(sq_bf[:, 0:128 * gn], tr_ps[:, 0:128 * gn])
                        s2 = sq_bf[:, 0:128 * gn]
                        nc.vector.tensor_tensor(out=dst, in0=s2, in1=s2, op=OP.mult)

                def stageO(t, p):
                    # output matmuls: W^T chunk stationary, V moving ->
                    # O accumulates in [i-part, 65] PSUM (col 64 = denominator).
                    # iota/vcum/out are laid out by processing position p.
                    _mark(f"O{t}")
                    i0 = 128 * t
                    u = p % 4
                    if u == 0:
                        live["ob"] = obp.tile([128, 260], fp32, name="ob")
                    ob = live["ob"]
                    jc_hi = t + 1 if causal else NJC
                    for jc in range(jc_hi):
                        nc.tensor.matmul(ob[:, 65 * u:65 * u + 65],
                                         wt_all[:, WT_OFF[t] + 128 * jc:WT_OFF[t] + 128 * jc + 128],
                                         v_s[:, 65 * jc:65 * jc + 65],
                                         start=(jc == 0), stop=(jc == jc_hi - 1))
                    if u == 3:
                        p0 = 4 * (p // 4)
                        # denom = psum col 64 (+65 stride) + iota; recip; then
                        # out = (num + vcum) * recip
                        dcol = bass.AP(ob[:].tensor, ob[:].offset + 64,
                                       [[260, 128], [65, 4]])
                        dtot = sp.tile([128, 4], fp32, name="dtot", tag="dtot")
                        nc.vector.tensor_tensor(out=dtot[:], in0=dcol,
                                                in1=iota_s[:, p0:p0 + 4], op=OP.add)
                        recip = sp.tile([128, 4], fp32, name="recip", tag="recip")
                        nc.vector.reciprocal(recip[:], dtot[:])
                        onum = bass.AP(ob[:].tensor, ob[:].offset,
                                       [[260, 128], [65, 4], [1, 64]])
                        osl = out_s[:, 64 * p0:64 * p0 + 256].rearrange(
                            "p (t d) -> p t d", d=64)
                        nc.vector.tensor_tensor(
                            out=osl, in0=onum,
                            in1=vcum_s[:, 64 * p0:64 * p0 + 256].rearrange(
                                "p (t d) -> p t d", d=64),
                            op=OP.add)
                        rb = bass.AP(recip[:].tensor, recip[:].offset,
                                     [[4, 128], [1, 4], [0, 64]])
                        nc.vector.tensor_tensor(out=osl, in0=osl, in1=rb, op=OP.mult)
                        dstap = bass.AP(o_d.ap().tensor, 64 * p0,
                                        [[NT * 64, 128], [1, 256]])
                        nc.sync.dma_start(out=dstap, in_=out_s[:, 64 * p0:64 * p0 + 256])

                # per-u issue order: oldest dependencies first so no engine
                # queue head-of-line blocks on freshly-issued producers; the
                # two transpose groups of C(t) are separated by B's matmuls so
                # the single trp buffer recycles without stalling PE.
                # Causal runs tiles biggest-first: the drain tile is tiny.
                NG = (NJC + TRG - 1) // TRG
                order = _tile_order(causal)
                DB = TUNE["depth_b"]
                DC, DO = DB + 1, DB + 2
                body = TUNE.get("body_order", 0)
                for u in range(NT + DO):
                    if u == 6:
                        nc.scalar.dma_start(out=vcum_s[:], in_=vcum_d.ap())
                        nc.scalar.dma_start(out=iota_s[:], in_=iota_d.ap())
                    def doA():
                        if u < NT:
                            stageA(order[u])
                    def doB():
                        if DB <= u < NT + DB:
                            stageB(order[u - DB])
                    def doC0():
                        if DC <= u < NT + DC:
                            stageC_group(order[u - DC], 0)
                    def doC1():
                        if DC <= u < NT + DC:
                            for g in range(1, NG):
                                stageC_group(order[u - DC], TRG * g)
                            live.pop(order[u - DC])
                    def doO():
                        if DO <= u < NT + DO:
                            stageO(order[u - DO], u - DO)
                    seqs = {
                        0: (doO, doC0, doB, doC1, doA),
                        1: (doA, doO, doC0, doB, doC1),
                        2: (doA, doB, doC0, doC1, doO),
                        3: (doA, doO, doC0, doC1, doB),
                        4: (doO, doA, doC0, doB, doC1),
                    }
                    for fn in seqs[body]:
                        fn()

    nc.compile()
    return nc


def _make_runner(nc, n_cores):
    import concourse.mybir as mybir
    import jax
    from jax.sharding import Mesh, PartitionSpec
    from jax.experimental.shard_map import shard_map
    from concourse.bass2jax import install_neuronx_cc_hook, _bass_exec_p, partition_id_tensor

    install_neuronx_cc_hook()
    partition_name = nc.partition_id_tensor.name if nc.partition_id_tensor else None
    in_names, out_names, out_avals, zero_outs = [], [], [], []
    for alloc in nc.m.functions[0].allocations:
        if not isinstance(alloc, mybir.MemoryLocationSet):
            continue
        name = alloc.memorylocations[0].name
        if alloc.kind == "ExternalInput":
            if name != partition_name:
                in_names.append(name)
        elif alloc.kind == "ExternalOutput":
            shape = tuple(alloc.tensor_shape)
            dtype = mybir.dt.np(alloc.dtype)
            out_names.append(name)
            out_avals.append(jax.core.ShapedArray(shape, dtype))
            zero_outs.append(np.zeros(shape, dtype))
    n_params = len(in_names)
    n_outs = len(out_avals)
    all_in_names = list(in_names) + list(out_names)
    if partition_name is not None:
        all_in_names.append(partition_name)

    def _body(*args):
        operands = list(args)
        if partition_name is not None:
            operands.append(partition_id_tensor())
        outs = _bass_exec_p.bind(
            *operands, out_avals=tuple(out_avals), in_names=tuple(all_in_names),
            out_names=tuple(out_names), lowering_input_output_aliases=(),
            sim_require_finite=True, sim_require_nnan=True, nc=nc)
        return tuple(outs)

    devices = jax.devices()[:n_cores]
    mesh = Mesh(np.asarray(devices), ("core",))
    in_specs = (PartitionSpec("core"),) * (n_params + n_outs)
    out_specs = (PartitionSpec("core"),) * n_outs
    jitted = jax.jit(shard_map(_body, mesh=mesh, in_specs=in_specs,
                               out_specs=out_specs, check_rep=False), keep_unused=True)

    def run(in_maps):
        concat_in = [np.concatenate([np.asarray(in_maps[c][n]) for c in range(n_cores)], axis=0)
                     for n in in_names]
        concat_zeros = [np.zeros((n_cores * z.shape[0], *z.shape[1:]), z.dtype) for z in zero_outs]
        outs = jitted(*concat_in, *concat_zeros)
        import jax as _jax
        _jax.block_until_ready(outs)
        return [{name: np.asarray(outs[i]).reshape(n_cores, *out_avals[i].shape)[c]
                 for i, name in enumerate(out_names)} for c in range(n_cores)]
    return run


def _get_runner(causal: bool):
    key = bool(causal)
    if key not in _CACHE:
        nc = _build_program(key)
        _CACHE[key] = _make_runner(nc, H)
    return _CACHE[key]


def _bf16(x):
    import ml_dtypes
    return np.ascontiguousarray(x).astype(ml_dtypes.bfloat16)


def _prep_head(q2, k2, v2, causal):
    """q2,k2,v2: [N, D] fp32 for one head. Returns per-core input dict."""
    qT = _bf16(q2.T)                                     # [64, N]
    kT = _bf16(k2.T)
    # v with ones col 64 (denominator)
    v3 = np.concatenate([v2, np.ones((N, 1), np.float32)], axis=1)  # [N, 65]
    v_r = _bf16(v3.reshape(NJC, 128, 65).transpose(1, 0, 2).reshape(128, NJC * 65))
    if causal:
        vc = np.cumsum(v2, axis=0, dtype=np.float64).astype(np.float32)
    else:
        vc = np.broadcast_to(v2.sum(axis=0, dtype=np.float64).astype(np.float32), (N, 64))
    vcum = vc.reshape(NT, 128, 64)[_tile_order(causal)]
    vcum = np.ascontiguousarray(vcum.transpose(1, 0, 2).reshape(128, NT * 64))
    return {"qT": qT, "kT": kT, "vr": v_r, "vcum": vcum}


def kernel(q, k, v, rpe_matrix, mask):
    causal = bool(np.asarray(mask).item()) if not isinstance(mask, (int, bool)) else bool(mask)
    q = np.asarray(q, dtype=np.float32)
    k = np.asarray(k, dtype=np.float32)
    v = np.asarray(v, dtype=np.float32)
    rpe = np.asarray(rpe_matrix, dtype=np.float32)

    RPW = 2560 if causal else 4608
    if causal:
        # u in [0, N-1]: rpe_rev[u] = rpe[2N-2-u] -> rows 2N-2 .. N-1
        rpe_rev = rpe[N - 1:2 * N - 1][::-1]             # [N, 64]
    else:
        rpe_rev = rpe[::-1]                              # [2N-1, 64]
    rpeT = np.zeros((64, RPW), dtype=np.float32)
    rpeT[:, :rpe_rev.shape[0]] = rpe_rev.T
    rpeT = _bf16(rpeT)

    a = np.arange(128, dtype=np.float32)[:, None]
    tt = np.asarray(_tile_order(causal), dtype=np.float32)[None, :]
    iota = (128 * tt + a + 1.0) if causal else np.full((128, NT), float(N), np.float32)
    iota = np.ascontiguousarray(iota.astype(np.float32))

    run = _get_runner(causal)
    in_maps = []
    for h in range(H):
        m = _prep_head(q[0, h], k[0, h], v[0, h], causal)
        m["rpeT"] = rpeT
        m["iota"] = iota
        in_maps.append(m)
    results = run(in_maps)
    # o_d layout: o[a, 64*p + d] = out[128*order[p] + a, d]
    inv = np.argsort(np.asarray(_tile_order(causal)))
    out = np.stack([
        results[h]["o"].reshape(128, NT, 64)[:, inv].transpose(1, 0, 2).reshape(N, 64)
        for h in range(H)])[None]
    return out.astype(np.float32)


if __name__ == "__main__":
    rng = np.random.default_rng(0)
    q = rng.standard_normal((B, H, N, D), dtype=np.float32)
    k = rng.standard_normal((B, H, N, D), dtype=np.float32)
    v = rng.standard_normal((B, H, N, D), dtype=np.float32)
    rpe = rng.standard_normal((2 * N - 1, D), dtype=np.float32)
    o = kernel(q, k, v, rpe, 1)
    print("out", o.shape, o.dtype, np.abs(o).mean())
